# revision 1
# baseline (speedup 1.0000x reference)
"""CTC loss (nn.CTCLoss, blank=0, reduction='mean', zero_infinity=True) for
T=160, B=64, C=6625, S=25 on 8 TRN2 NeuronCores.

Sharding: data-parallel over batch — 8 of the 64 samples per core.

Algorithm (mathematically identical to the log-domain reference): the CTC
forward DP runs in the probability domain with periodic rescaling.  With
p[t,s] = exp(score of extended-target symbol s at time t) and
q = p * skip_mask, each step is

    alpha_new[s] = alpha[s-2]*q[t,s] + alpha[s-1]*p[t,s] + alpha[s]*p[t,s]

computed as TWO Vector-engine ops on an [8, 51, 3] tile: one elementwise
multiply of the overlapped 3-tap view of alpha against a pre-interleaved
(q,p,p) coefficient array, then a strided reduce_sum over the tap axis.
Every 8 steps the per-sample sum is folded out as log(scale).  Only the 51
extended-target class columns are gathered from the predictions shard
(indirect DMA); the other 6574 classes are never read.
"""

import numpy as np

import concourse.bacc as bacc
import concourse.bass as bass
import concourse.mybir as mybir
import concourse.tile as tile
from concourse.bass_utils import run_bass_kernel_spmd

T = 160
B = 64
C = 6625
S = 25
L = 2 * S + 1  # 51
NCORES = 8
BLOC = B // NCORES  # 8 samples per core
NORM_EVERY = 8
NG = (BLOC * L + 127) // 128  # 4 gather blocks of 128 rows (408 pad to 512)

F32 = mybir.dt.float32
I32 = mybir.dt.int32
ALU = mybir.AluOpType
ACTF = mybir.ActivationFunctionType
AXIS = mybir.AxisListType


def _combined_act_tables(module_arch):
    """Force Exp and Ln into one table set (one ~2.7us ACT_TABLE_LOAD instead
    of two).  Set names/positions are preserved (ids are positional); sets
    other than the combined exp+ln one just stop advertising Exp/Ln."""
    tables = dict(_orig_get_activation_tables(module_arch))
    both = {ACTF.Exp, ACTF.Ln}
    combined = [n for n, fns in tables.items() if both <= fns]
    if combined:
        keep = combined[0]
        for n in list(tables):
            if n != keep and (tables[n] & both):
                tables[n] = tables[n] - both
    return tables


_orig_get_activation_tables = bacc.get_activation_tables


def build_nc(loop_T: int = T) -> bass.Bass:
    bacc.get_activation_tables = _combined_act_tables
    nc = bacc.Bacc("TRN2", target_bir_lowering=False)

    preds = nc.dram_tensor("preds", [BLOC * C, T], F32, kind="ExternalInput")
    gidx_d = nc.dram_tensor("gidx", [128, NG], I32, kind="ExternalInput")
    maskc_d = nc.dram_tensor("maskcol", [128, NG], F32, kind="ExternalInput")
    oneh_d = nc.dram_tensor("onehot", [BLOC, L], F32, kind="ExternalInput")
    out_d = nc.dram_tensor("out2", [BLOC, 2], F32, kind="ExternalOutput")
    pscr_p = nc.dram_tensor("pscr_p", [128 * NG, T], F32)  # internal bounce
    pscr_q = nc.dram_tensor("pscr_q", [128 * NG, T], F32)

    n_scales = (T - 2) // NORM_EVERY  # t=7,15,...,151 -> 19 rescales
    with tile.TileContext(nc) as tc:
        with (
            tc.tile_pool(name="big", bufs=1) as bigp,
            tc.tile_pool(name="small", bufs=1) as smallp,
            tc.tile_pool(name="tmp", bufs=2) as tmpp,
        ):
            G = bigp.tile([128, NG, T], F32, tag="G")
            Gp = bigp.tile([128, NG, T], F32, tag="Gp")
            Gq = bigp.tile([128, NG, T], F32, tag="Gq")
            # PPQ[b, l, j, t] = (q, p, p)[j] at (b, l, t); chunked along t so
            # the loop can start as soon as the first chunk's DMAs land.
            TCH = 40
            NCH = (T + TCH - 1) // TCH
            PPQs = [
                bigp.tile([BLOC, L, 3, TCH], F32, tag=f"PPQ{c}", name=f"PPQ{c}")
                for c in range(NCH)
            ]

            gidx = smallp.tile([128, NG], I32, tag="gidx")
            maskc = smallp.tile([128, NG], F32, tag="maskc")
            oneh = smallp.tile([BLOC, L], F32, tag="oneh")
            X = smallp.tile([BLOC, L + 2], F32, tag="X")
            Y = smallp.tile([BLOC, L + 2], F32, tag="Y")
            scales = smallp.tile([BLOC, n_scales + 1], F32, tag="scales")
            logs = smallp.tile([BLOC, n_scales + 1], F32, tag="logs")
            rcol = smallp.tile([BLOC, 1], F32, tag="rcol")
            out_s = smallp.tile([BLOC, 2], F32, tag="out_s")

            nc.sync.dma_start(out=gidx[:, :], in_=gidx_d[:, :])
            nc.sync.dma_start(out=maskc[:, :], in_=maskc_d[:, :])
            nc.sync.dma_start(out=oneh[:, :], in_=oneh_d[:, :])

            # Gather row-per-partition: G[p, j, :] = preds[gidx[p, j], :]
            for j in range(NG):
                nc.gpsimd.indirect_dma_start(
                    out=G[:, j, :],
                    out_offset=None,
                    in_=preds[:, :],
                    in_offset=bass.IndirectOffsetOnAxis(ap=gidx[:, j : j + 1], axis=0),
                )
            # Pipeline exp/mask/bounce per t-chunk so the DP loop can start
            # once chunk 0 lands; chunks 1..3 overlap with the loop.
            # pscr rows are r = j*128 + p  (flat b-major row id b*L + l).
            for c in range(NCH):
                cs = slice(c * TCH, (c + 1) * TCH)
                nc.scalar.activation(Gp[:, :, cs], G[:, :, cs], ACTF.Exp)
                for j in range(NG):
                    # per-partition scalar multiply on the (otherwise idle)
                    # ACT engine, keeping the Vector engine free for the loop
                    nc.scalar.mul(
                        Gq[:, j, cs], Gp[:, j, cs], maskc[:, j : j + 1]
                    )
                out_ap_p = bass.AP(
                    pscr_p, c * TCH, [[T, 128], [128 * T, NG], [1, TCH]]
                )
                out_ap_q = bass.AP(
                    pscr_q, c * TCH, [[T, 128], [128 * T, NG], [1, TCH]]
                )
                nc.sync.dma_start(out=out_ap_p, in_=Gp[:, :, cs])
                nc.sync.dma_start(out=out_ap_q, in_=Gq[:, :, cs])
                in_p = bass.AP(pscr_p, c * TCH, [[L * T, BLOC], [T, L], [1, TCH]])
                in_q = bass.AP(pscr_q, c * TCH, [[L * T, BLOC], [T, L], [1, TCH]])
                nc.sync.dma_start(out=PPQs[c][:, :, 0, :], in_=in_q)
                nc.sync.dma_start(out=PPQs[c][:, :, 1, :], in_=in_p)
                nc.sync.dma_start(out=PPQs[c][:, :, 2, :], in_=in_p)

            # alpha0: [p(0,0), p(0,1), 0, ...] in padded cols 2:4 of X
            nc.vector.memset(X[:, :], 0.0)
            nc.vector.memset(Y[:, :], 0.0)
            nc.vector.tensor_copy(X[:, 2:4], PPQs[0][:, 0:2, 1, 0])

            cur, nxt = X, Y
            apply_norm = False
            for t in range(1, loop_T):
                ppq_t = PPQs[t // TCH][:, :, :, t % TCH]
                xap = cur[:, :]
                xxx = bass.AP(xap.tensor, xap.offset, [xap.ap[0], [1, L], [1, 3]])

                M = tmpp.tile([BLOC, L, 3], F32, tag="M")
                is_norm = t % NORM_EVERY == NORM_EVERY - 1 and t < T - 1
                k = t // NORM_EVERY
                if apply_norm or is_norm:
                    # stt form: optional rescale via scalar, and on norm steps
                    # the accum_out gives sum(M) = sum(alpha_new) for free.
                    # (tensor_tensor_reduce would fuse this cheaper per the
                    # cost model but fails on HW with these overlapped APs.)
                    nc.vector.scalar_tensor_tensor(
                        out=M[:, :, :], in0=xxx,
                        scalar=rcol[:, :] if apply_norm else 1.0, in1=ppq_t,
                        op0=ALU.mult, op1=ALU.mult,
                        accum_out=scales[:, k : k + 1] if is_norm else None,
                    )
                    apply_norm = False
                else:
                    nc.vector.tensor_tensor(
                        out=M[:, :, :], in0=xxx, in1=ppq_t, op=ALU.mult
                    )
                nc.vector.tensor_reduce(
                    out=nxt[:, 2 : L + 2], in_=M[:, :, :], axis=AXIS.X, op=ALU.add
                )
                if is_norm:
                    nc.vector.reciprocal(out=rcol[:, :], in_=scales[:, k : k + 1])
                    apply_norm = True
                cur, nxt = nxt, cur

            # Final-state sum (one more rescale so dot is well-conditioned),
            # then dot = sum_s (alpha[s]/s_fin) * onehot[s].  log(dot) happens
            # on the host: the ACT Ln table clamps inputs below ~1e-20 and dot
            # can be that small; the device only ever Ln's the window sums,
            # which are safely in range.
            nc.vector.tensor_reduce(
                out=scales[:, n_scales : n_scales + 1], in_=cur[:, 2 : L + 2],
                axis=AXIS.X, op=ALU.add,
            )
            nc.vector.reciprocal(out=rcol[:, :], in_=scales[:, n_scales : n_scales + 1])
            z2 = tmpp.tile([BLOC, L], F32, tag="z2")
            nc.vector.scalar_tensor_tensor(
                out=z2[:, :], in0=cur[:, 2 : L + 2], scalar=rcol[:, :], in1=oneh[:, :],
                op0=ALU.mult, op1=ALU.mult,
                accum_out=out_s[:, 1:2],
            )
            # out_s[:, 0] = sum_k log(scale_k) + log(s_fin)
            nc.scalar.activation(logs[:, :], scales[:, :], ACTF.Ln)
            nc.vector.tensor_reduce(
                out=out_s[:, 0:1], in_=logs[:, :], axis=AXIS.X, op=ALU.add
            )
            nc.sync.dma_start(out=out_d[:, :], in_=out_s[:, :])

    try:
        nc.finalize()
    finally:
        bacc.get_activation_tables = _orig_get_activation_tables
    return nc


def host_prep(predictions, targets, target_lengths):
    """Host-side shard + index prep. Returns per-core input maps."""
    predictions = np.asarray(predictions, dtype=np.float32)
    targets = np.asarray(targets)
    target_lengths = np.asarray(target_lengths)

    ext = np.zeros((B, L), dtype=np.int64)
    ext[:, 1::2] = targets
    mask01 = np.zeros((B, L), dtype=np.float32)
    mask01[:, 3::2] = (targets[:, 1:] != targets[:, :-1]).astype(np.float32)
    onehot = np.zeros((B, L), dtype=np.float32)
    idx = (2 * target_lengths).astype(np.int64)
    onehot[np.arange(B), idx] = 1.0
    onehot[np.arange(B), idx - 1] = 1.0

    in_maps = []
    for k in range(NCORES):
        bsl = slice(k * BLOC, (k + 1) * BLOC)
        # [T, BLOC, C] -> [BLOC, C, T] contiguous -> flat [BLOC*C, T]
        pshard = np.ascontiguousarray(
            predictions[:, bsl, :].transpose(1, 2, 0)
        ).reshape(BLOC * C, T)
        gidx = (
            np.arange(BLOC, dtype=np.int64)[:, None] * C + ext[bsl]
        ).astype(np.int32)
        # b-major flat row list, padded to 128*NG, as [128, NG] column-blocks
        gflat = np.zeros(128 * NG, dtype=np.int32)
        gflat[: BLOC * L] = gidx.reshape(-1)
        mflat = np.zeros(128 * NG, dtype=np.float32)
        mflat[: BLOC * L] = mask01[bsl].reshape(-1)
        in_maps.append(
            {
                "preds": pshard,
                "gidx": gflat.reshape(NG, 128).T.copy(),
                "maskcol": mflat.reshape(NG, 128).T.copy(),
                "onehot": onehot[bsl],
            }
        )
    return in_maps


_NC_CACHE = {}


def kernel(predictions, targets, target_lengths):
    if "nc" not in _NC_CACHE:
        _NC_CACHE["nc"] = build_nc()
    nc = _NC_CACHE["nc"]

    in_maps = host_prep(predictions, targets, target_lengths)
    res = run_bass_kernel_spmd(nc, in_maps, core_ids=list(range(NCORES)))
    return finish(res.results, target_lengths)


def finish(results, target_lengths):
    out2 = np.concatenate([r["out2"].reshape(BLOC, 2) for r in results])
    slogsum, dot = out2[:, 0], out2[:, 1]
    with np.errstate(divide="ignore"):
        nll = -(np.log(dot.astype(np.float32)).astype(np.float32) + slogsum)
    lengths = np.asarray(target_lengths).astype(np.float32)
    per = np.where(nll >= 1e29, np.float32(0.0), nll / lengths)
    return np.array(per.mean(), dtype=np.float32)



# revision 6
# speedup vs baseline: 2.1030x; 2.1030x over previous
"""CTC loss (nn.CTCLoss, blank=0, reduction='mean', zero_infinity=True) for
T=160, B=64, C=6625, S=25 on 8 TRN2 NeuronCores.

Sharding: data-parallel over batch — 8 of the 64 samples per core.

Algorithm: probability-domain CTC forward DP with periodic rescaling, run
BIDIRECTIONALLY to halve the sequential chain: rows 0-7 of the state tile run
alpha forward from t=0, rows 8-15 run beta backward from t=159 with the
extended-target axis reversed, which makes both recurrences the identical
3-tap stencil

    state_new[s] = state[s-2]*c0[s] + state[s-1]*c1[s] + state[s]*c2[s]

computed as one [16, 51, 3] Vector-engine multiply of the overlapped 3-tap
state view against per-iteration coefficients, then a strided reduce over the
tap axis.  The chains meet at t=80 where loss_b = dot(alpha_80, beta_80)
(invariant in the meeting point).  80 iterations instead of 159.

The host packs the coefficients: for each iteration slot i it gathers the 51
extended-target class scores (plus the skip mask as -1e30 fills and the bwd
time/axis reversal — pure selection/layout, no float math) into
pil[i, row, l, tap]; the device exponentiates chunks of slots on the Scalar
engine, pipelined ahead of the DP loop.  Every 8 steps the running sum is
folded out through a fast approximate reciprocal whose exact value is shipped
to the host, so the rescale cancels exactly in the final log-domain combine.
"""

import numpy as np

import concourse.bacc as bacc
import concourse.bass as bass
import concourse.mybir as mybir
import concourse.tile as tile
from concourse.bass_utils import run_bass_kernel_spmd

T = 160
B = 64
C = 6625
S = 25
L = 2 * S + 1  # 51
NCORES = 8
BLOC = B // NCORES  # 8 samples per core
ROWS = 2 * BLOC  # fwd + bwd chains
NITER = 81  # coefficient slots i=0..80; DP loop runs i=1..80
NORM_EVERY = 8
NNORM = 10  # rescales at i = 7, 15, ..., 79
NEG = -1e30
CHUNKS = [6, 12, 15, 24, 24]  # slot-chunk sizes for the DMA/exp pipeline

F32 = mybir.dt.float32
ALU = mybir.AluOpType
ACTF = mybir.ActivationFunctionType
AXIS = mybir.AxisListType


def build_nc() -> bass.Bass:
    nc = bacc.Bacc("TRN2", target_bir_lowering=False)

    pil_d = nc.dram_tensor("pil", [NITER, ROWS, L * 3], F32, kind="ExternalInput")
    oneh_d = nc.dram_tensor("onehotr", [BLOC, L], F32, kind="ExternalInput")
    out_d = nc.dram_tensor("outv", [ROWS, NNORM + 2], F32, kind="ExternalOutput")
    st_d = nc.dram_tensor("state", [ROWS, L], F32, kind="ExternalOutput")

    with tile.TileContext(nc) as tc:
        with (
            tc.tile_pool(name="big", bufs=1) as bigp,
            tc.tile_pool(name="small", bufs=1) as smallp,
            tc.tile_pool(name="tmp", bufs=2) as tmpp,
        ):
            pilin = bigp.tile([ROWS, NITER, L * 3], F32, tag="pilin")
            PPQ = bigp.tile([ROWS, NITER, L * 3], F32, tag="PPQ")

            X = smallp.tile([ROWS, L + 2], F32, tag="X")
            Y = smallp.tile([ROWS, L + 2], F32, tag="Y")
            rall = smallp.tile([ROWS, NNORM + 2], F32, tag="rall")
            ssum = smallp.tile([ROWS, NNORM], F32, tag="ssum")
            dummy = smallp.tile([1, 1], F32, tag="dummy")

            # Hoist the Exp act-table load to t~0: a dependency-free dummy
            # activation makes the scheduler place ACT_TABLE_LOAD before any
            # data arrives.
            nc.vector.memset(dummy[:, :], 0.0)
            nc.scalar.activation(dummy[:, :], dummy[:, :], ACTF.Exp)

            nc.vector.memset(X[:, :], 0.0)
            nc.vector.memset(Y[:, :], 0.0)
            nc.vector.memset(rall[:, :], 0.0)

            # bwd init: G_0[sig] = reversed end-state indicator (ACT queue so
            # it overlaps the chunk-0 coefficient DMA on the SP queue).
            nc.scalar.dma_start(out=X[BLOC:ROWS, 2 : L + 2], in_=oneh_d[:, :])

            # Coefficient pipeline: DMA slot-chunk -> exp on ACT.
            s0 = 0
            for ns in CHUNKS:
                in_ap = bass.AP(
                    pil_d,
                    s0 * ROWS * L * 3,
                    [[L * 3, ROWS], [ROWS * L * 3, ns], [1, L * 3]],
                )
                nc.sync.dma_start(out=pilin[:, s0 : s0 + ns, :], in_=in_ap)
                nc.scalar.activation(
                    PPQ[:, s0 : s0 + ns, :], pilin[:, s0 : s0 + ns, :], ACTF.Exp
                )
                s0 += ns

            # fwd init: alpha_0[s=0,1] = p(t=0, l=0,1) = exp'd slot-0 tap 1.
            a0 = PPQ[0:BLOC, 0, :]
            a0v = bass.AP(a0.tensor, a0.offset + 1, [a0.ap[0], [3, 2]])
            nc.vector.tensor_copy(X[0:BLOC, 2:4], a0v)

            cur, nxt = X, Y
            apply_norm = False
            for i in range(1, NITER):
                pp = PPQ[:, i, :]
                ppq_i = bass.AP(pp.tensor, pp.offset, [pp.ap[0], [3, L], [1, 3]])
                xap = cur[:, :]
                xxx = bass.AP(xap.tensor, xap.offset, [xap.ap[0], [1, L], [1, 3]])

                M = tmpp.tile([ROWS, L, 3], F32, tag="M")
                is_norm = i % NORM_EVERY == NORM_EVERY - 1
                k = i // NORM_EVERY
                if apply_norm or is_norm:
                    nc.vector.scalar_tensor_tensor(
                        out=M[:, :, :],
                        in0=xxx,
                        scalar=rall[:, k - 1 : k] if apply_norm else 1.0,
                        in1=ppq_i,
                        op0=ALU.mult,
                        op1=ALU.mult,
                        accum_out=ssum[:, k : k + 1] if is_norm else None,
                    )
                    apply_norm = False
                else:
                    nc.vector.tensor_tensor(
                        out=M[:, :, :], in0=xxx, in1=ppq_i, op=ALU.mult
                    )
                nc.vector.tensor_reduce(
                    out=nxt[:, 2 : L + 2], in_=M[:, :, :], axis=AXIS.X, op=ALU.add
                )
                if is_norm:
                    nc.vector.reciprocal_approx_fast(
                        out=rall[:, k : k + 1], in_=ssum[:, k : k + 1]
                    )
                    apply_norm = True
                cur, nxt = nxt, cur

            # Ship the meeting-point states; the host computes the tiny
            # 51-element dot(alpha_80, reverse(beta_80)) per sample (the
            # pairing crosses partitions, which DVE lanes cannot).
            nc.sync.dma_start(out=st_d[:, :], in_=cur[:, 2 : L + 2])
            nc.scalar.dma_start(out=out_d[:, :], in_=rall[:, :])

    nc.finalize()
    return nc


def host_prep(predictions, targets, target_lengths):
    """Host-side shard + coefficient-layout prep (gather/select/reverse only;
    all float math on the predictions happens on device). Returns per-core
    input maps."""
    predictions = np.asarray(predictions, dtype=np.float32)
    targets = np.asarray(targets)
    target_lengths = np.asarray(target_lengths)

    ext = np.zeros((B, L), dtype=np.int64)
    ext[:, 1::2] = targets
    skip = np.zeros((B, L), dtype=bool)
    skip[:, 3::2] = targets[:, 1:] != targets[:, :-1]

    # sc[t, b, l] = predictions[t, b, ext[b, l]]; q = skip-masked scores
    sc = np.take_along_axis(
        predictions, np.broadcast_to(ext[None], (T, B, L)), axis=2
    )
    q = np.where(skip[None], sc, np.float32(NEG))

    pil = np.full((NITER, 2 * B, L, 3), NEG, dtype=np.float32)
    # fwd rows (slot i = time t=i): taps (q[l], p[l], p[l])
    pil[:, :B, :, 0] = q[:NITER]
    pil[:, :B, :, 1] = sc[:NITER]
    pil[:, :B, :, 2] = sc[:NITER]
    # bwd rows (slot i = time t=160-i, i=1..79), state axis reversed:
    # taps at sigma: (q[52-sig], p[51-sig], p[50-sig]); l out of range -> NEG
    sig = np.arange(L)
    for i in range(1, NITER - 1):
        t = T - i
        l0, l1, l2 = 52 - sig, 51 - sig, 50 - sig
        v0 = np.where(l0[None] < L, q[t][:, np.minimum(l0, L - 1)], np.float32(NEG))
        v1 = np.where(l1[None] < L, sc[t][:, np.minimum(l1, L - 1)], np.float32(NEG))
        pil[i, B:, :, 0] = v0
        pil[i, B:, :, 1] = v1
        pil[i, B:, :, 2] = sc[t][:, l2]
    # bwd slot 80: identity step (taps exp -> (0, 0, 1))
    pil[NITER - 1, B:, :, 2] = 0.0

    idx = (2 * target_lengths).astype(np.int64)
    oneh_rev = np.zeros((B, L), dtype=np.float32)
    oneh_rev[np.arange(B), (L - 1) - idx] = 1.0
    oneh_rev[np.arange(B), (L - 1) - (idx - 1)] = 1.0

    in_maps = []
    for c in range(NCORES):
        bsl = slice(c * BLOC, (c + 1) * BLOC)
        bsl2 = slice(B + c * BLOC, B + (c + 1) * BLOC)
        pshard = np.concatenate([pil[:, bsl], pil[:, bsl2]], axis=1)  # [81,16,51,3]
        in_maps.append(
            {
                "pil": np.ascontiguousarray(pshard).reshape(NITER, ROWS, L * 3),
                "onehotr": oneh_rev[bsl],
            }
        )
    return in_maps


_NC_CACHE = {}


def kernel(predictions, targets, target_lengths):
    if "nc" not in _NC_CACHE:
        _NC_CACHE["nc"] = build_nc()
    nc = _NC_CACHE["nc"]

    in_maps = host_prep(predictions, targets, target_lengths)
    res = run_bass_kernel_spmd(nc, in_maps, core_ids=list(range(NCORES)))
    return finish(res.results, target_lengths)


def finish(results, target_lengths):
    nlls = []
    with np.errstate(divide="ignore"):
        for r in results:
            outv = r["outv"].reshape(ROWS, NNORM + 2)
            st = r["state"].reshape(ROWS, L)
            slog = -np.log(outv[:, :NNORM]).sum(axis=1)  # -sum log rcol per row
            dot = (st[:BLOC] * st[BLOC:, ::-1]).sum(axis=1)
            nlls.append(-(np.log(dot) + slog[:BLOC] + slog[BLOC:]))
    nll = np.concatenate(nlls).astype(np.float32)
    lengths = np.asarray(target_lengths).astype(np.float32)
    per = np.where(nll >= 1e29, np.float32(0.0), nll / lengths)
    return np.array(per.mean(), dtype=np.float32)


# revision 7
# speedup vs baseline: 2.1088x; 1.0028x over previous
"""CTC loss (nn.CTCLoss, blank=0, reduction='mean', zero_infinity=True) for
T=160, B=64, C=6625, S=25 on 8 TRN2 NeuronCores.

Sharding: data-parallel over batch — 8 of the 64 samples per core.

Algorithm: probability-domain CTC forward DP with periodic rescaling, run
BIDIRECTIONALLY to halve the sequential chain: rows 0-7 of the state tile run
alpha forward from t=0, rows 8-15 run beta backward from t=159 with the
extended-target axis reversed, which makes both recurrences the identical
3-tap stencil

    state_new[s] = state[s-2]*c0[s] + state[s-1]*c1[s] + state[s]*c2[s]

computed as one [16, 51, 3] Vector-engine multiply of the overlapped 3-tap
state view against per-iteration coefficients, then a strided reduce over the
tap axis.  The chains meet at t=80 where loss_b = dot(alpha_80, beta_80)
(invariant in the meeting point).  80 iterations instead of 159.

The host packs the coefficients: for each iteration slot i it gathers the 51
extended-target class scores (plus the skip mask as -1e30 fills and the bwd
time/axis reversal — pure selection/layout, no float math) into
pil[i, row, l, tap]; the device exponentiates chunks of slots on the Scalar
engine, pipelined ahead of the DP loop.  Every 8 steps the running sum is
folded out through a fast approximate reciprocal whose exact value is shipped
to the host, so the rescale cancels exactly in the final log-domain combine.
"""

import numpy as np

import concourse.bacc as bacc
import concourse.bass as bass
import concourse.mybir as mybir
import concourse.tile as tile
from concourse.bass_utils import run_bass_kernel_spmd

T = 160
B = 64
C = 6625
S = 25
L = 2 * S + 1  # 51
NCORES = 8
BLOC = B // NCORES  # 8 samples per core
ROWS = 2 * BLOC  # fwd + bwd chains
NITER = 81  # coefficient slots i=0..80; DP loop runs i=1..80
NORM_EVERY = 16
NNORM = 5  # rescales at i = 15, 31, ..., 79
NEG = -1e30
CHUNKS = [4, 8, 12, 17, 20, 20]  # slot-chunk sizes for the DMA/exp pipeline

F32 = mybir.dt.float32
ALU = mybir.AluOpType
ACTF = mybir.ActivationFunctionType
AXIS = mybir.AxisListType


def build_nc() -> bass.Bass:
    nc = bacc.Bacc("TRN2", target_bir_lowering=False)

    pil_d = nc.dram_tensor("pil", [NITER, ROWS, L * 3], F32, kind="ExternalInput")
    oneh_d = nc.dram_tensor("onehotr", [BLOC, L], F32, kind="ExternalInput")
    out_d = nc.dram_tensor("outv", [ROWS, NNORM + 2], F32, kind="ExternalOutput")
    st_d = nc.dram_tensor("state", [ROWS, L], F32, kind="ExternalOutput")

    with tile.TileContext(nc) as tc:
        with (
            tc.tile_pool(name="big", bufs=1) as bigp,
            tc.tile_pool(name="small", bufs=1) as smallp,
            tc.tile_pool(name="tmp", bufs=2) as tmpp,
        ):
            pilin = bigp.tile([ROWS, NITER, L * 3], F32, tag="pilin")
            PPQ = bigp.tile([ROWS, NITER, L * 3], F32, tag="PPQ")

            X = smallp.tile([ROWS, L + 2], F32, tag="X")
            Y = smallp.tile([ROWS, L + 2], F32, tag="Y")
            rall = smallp.tile([ROWS, NNORM + 2], F32, tag="rall")
            ssum = smallp.tile([ROWS, NNORM], F32, tag="ssum")
            dummy = smallp.tile([1, 1], F32, tag="dummy")

            # Hoist the Exp act-table load to t~0: a dependency-free dummy
            # activation makes the scheduler place ACT_TABLE_LOAD before any
            # data arrives.
            nc.vector.memset(dummy[:, :], 0.0)
            nc.scalar.activation(dummy[:, :], dummy[:, :], ACTF.Exp)

            nc.vector.memset(X[:, :], 0.0)
            nc.vector.memset(Y[:, :], 0.0)
            nc.vector.memset(rall[:, :], 0.0)

            # bwd init: G_0[sig] = reversed end-state indicator (ACT queue so
            # it overlaps the chunk-0 coefficient DMA on the SP queue).
            nc.scalar.dma_start(out=X[BLOC:ROWS, 2 : L + 2], in_=oneh_d[:, :])

            # Coefficient pipeline: DMA slot-chunk -> exp on ACT.
            s0 = 0
            for ns in CHUNKS:
                in_ap = bass.AP(
                    pil_d,
                    s0 * ROWS * L * 3,
                    [[L * 3, ROWS], [ROWS * L * 3, ns], [1, L * 3]],
                )
                nc.sync.dma_start(out=pilin[:, s0 : s0 + ns, :], in_=in_ap)
                nc.scalar.activation(
                    PPQ[:, s0 : s0 + ns, :], pilin[:, s0 : s0 + ns, :], ACTF.Exp
                )
                s0 += ns

            # fwd init: alpha_0[s=0,1] = p(t=0, l=0,1) = exp'd slot-0 tap 1.
            a0 = PPQ[0:BLOC, 0, :]
            a0v = bass.AP(a0.tensor, a0.offset + 1, [a0.ap[0], [3, 2]])
            nc.vector.tensor_copy(X[0:BLOC, 2:4], a0v)

            cur, nxt = X, Y
            apply_norm = False
            for i in range(1, NITER):
                pp = PPQ[:, i, :]
                ppq_i = bass.AP(pp.tensor, pp.offset, [pp.ap[0], [3, L], [1, 3]])
                xap = cur[:, :]
                xxx = bass.AP(xap.tensor, xap.offset, [xap.ap[0], [1, L], [1, 3]])

                M = tmpp.tile([ROWS, L, 3], F32, tag="M")
                is_norm = i % NORM_EVERY == NORM_EVERY - 1
                k = i // NORM_EVERY
                if apply_norm or is_norm:
                    nc.vector.scalar_tensor_tensor(
                        out=M[:, :, :],
                        in0=xxx,
                        scalar=rall[:, k - 1 : k] if apply_norm else 1.0,
                        in1=ppq_i,
                        op0=ALU.mult,
                        op1=ALU.mult,
                        accum_out=ssum[:, k : k + 1] if is_norm else None,
                    )
                    apply_norm = False
                else:
                    nc.vector.tensor_tensor(
                        out=M[:, :, :], in0=xxx, in1=ppq_i, op=ALU.mult
                    )
                nc.vector.tensor_reduce(
                    out=nxt[:, 2 : L + 2], in_=M[:, :, :], axis=AXIS.X, op=ALU.add
                )
                if is_norm:
                    nc.vector.reciprocal_approx_fast(
                        out=rall[:, k : k + 1], in_=ssum[:, k : k + 1]
                    )
                    apply_norm = True
                cur, nxt = nxt, cur

            # Ship the meeting-point states; the host computes the tiny
            # 51-element dot(alpha_80, reverse(beta_80)) per sample (the
            # pairing crosses partitions, which DVE lanes cannot).
            nc.sync.dma_start(out=st_d[:, :], in_=cur[:, 2 : L + 2])
            nc.scalar.dma_start(out=out_d[:, :], in_=rall[:, :])

    nc.finalize()
    return nc


def host_prep(predictions, targets, target_lengths):
    """Host-side shard + coefficient-layout prep (gather/select/reverse only;
    all float math on the predictions happens on device). Returns per-core
    input maps."""
    predictions = np.asarray(predictions, dtype=np.float32)
    targets = np.asarray(targets)
    target_lengths = np.asarray(target_lengths)

    ext = np.zeros((B, L), dtype=np.int64)
    ext[:, 1::2] = targets
    skip = np.zeros((B, L), dtype=bool)
    skip[:, 3::2] = targets[:, 1:] != targets[:, :-1]

    # sc[t, b, l] = predictions[t, b, ext[b, l]]; q = skip-masked scores
    sc = np.take_along_axis(
        predictions, np.broadcast_to(ext[None], (T, B, L)), axis=2
    )
    q = np.where(skip[None], sc, np.float32(NEG))

    pil = np.full((NITER, 2 * B, L, 3), NEG, dtype=np.float32)
    # fwd rows (slot i = time t=i): taps (q[l], p[l], p[l])
    pil[:, :B, :, 0] = q[:NITER]
    pil[:, :B, :, 1] = sc[:NITER]
    pil[:, :B, :, 2] = sc[:NITER]
    # bwd rows (slot i = time t=160-i, i=1..79), state axis reversed:
    # taps at sigma: (q[52-sig], p[51-sig], p[50-sig]); l out of range -> NEG
    sig = np.arange(L)
    for i in range(1, NITER - 1):
        t = T - i
        l0, l1, l2 = 52 - sig, 51 - sig, 50 - sig
        v0 = np.where(l0[None] < L, q[t][:, np.minimum(l0, L - 1)], np.float32(NEG))
        v1 = np.where(l1[None] < L, sc[t][:, np.minimum(l1, L - 1)], np.float32(NEG))
        pil[i, B:, :, 0] = v0
        pil[i, B:, :, 1] = v1
        pil[i, B:, :, 2] = sc[t][:, l2]
    # bwd slot 80: identity step (taps exp -> (0, 0, 1))
    pil[NITER - 1, B:, :, 2] = 0.0

    idx = (2 * target_lengths).astype(np.int64)
    oneh_rev = np.zeros((B, L), dtype=np.float32)
    oneh_rev[np.arange(B), (L - 1) - idx] = 1.0
    oneh_rev[np.arange(B), (L - 1) - (idx - 1)] = 1.0

    in_maps = []
    for c in range(NCORES):
        bsl = slice(c * BLOC, (c + 1) * BLOC)
        bsl2 = slice(B + c * BLOC, B + (c + 1) * BLOC)
        pshard = np.concatenate([pil[:, bsl], pil[:, bsl2]], axis=1)  # [81,16,51,3]
        in_maps.append(
            {
                "pil": np.ascontiguousarray(pshard).reshape(NITER, ROWS, L * 3),
                "onehotr": oneh_rev[bsl],
            }
        )
    return in_maps


_NC_CACHE = {}


def kernel(predictions, targets, target_lengths):
    if "nc" not in _NC_CACHE:
        _NC_CACHE["nc"] = build_nc()
    nc = _NC_CACHE["nc"]

    in_maps = host_prep(predictions, targets, target_lengths)
    res = run_bass_kernel_spmd(nc, in_maps, core_ids=list(range(NCORES)))
    return finish(res.results, target_lengths)


def finish(results, target_lengths):
    nlls = []
    with np.errstate(divide="ignore"):
        for r in results:
            outv = r["outv"].reshape(ROWS, NNORM + 2)
            st = r["state"].reshape(ROWS, L)
            slog = -np.log(outv[:, :NNORM]).sum(axis=1)  # -sum log rcol per row
            dot = (st[:BLOC] * st[BLOC:, ::-1]).sum(axis=1)
            nlls.append(-(np.log(dot) + slog[:BLOC] + slog[BLOC:]))
    nll = np.concatenate(nlls).astype(np.float32)
    lengths = np.asarray(target_lengths).astype(np.float32)
    per = np.where(nll >= 1e29, np.float32(0.0), nll / lengths)
    return np.array(per.mean(), dtype=np.float32)


# revision 12
# speedup vs baseline: 2.3358x; 1.1076x over previous
"""CTC loss (nn.CTCLoss, blank=0, reduction='mean', zero_infinity=True) for
T=160, B=64, C=6625, S=25 on 8 TRN2 NeuronCores.

Sharding: data-parallel over batch — 8 of the 64 samples per core.

Algorithm: probability-domain CTC forward DP with periodic rescaling, run
BIDIRECTIONALLY to halve the sequential chain: rows 0-7 of the state tile run
alpha forward from t=0, rows 8-15 run beta backward from t=159 with the
extended-target axis reversed, which makes both recurrences the identical
3-tap stencil

    state_new[s] = state[s-2]*c0[s] + state[s-1]*c1[s] + state[s]*c2[s]

computed as one [16, 51, 3] Vector-engine multiply of the overlapped 3-tap
state view against per-iteration coefficients, then a strided reduce over the
tap axis.  The chains meet at t=80 where loss_b = dot(alpha_80, beta_80)
(invariant in the meeting point).  80 iterations instead of 159.

The host packs the coefficients: for each iteration slot i it gathers the 51
extended-target class scores (plus the skip mask as -1e30 fills and the bwd
time/axis reversal — pure selection/layout, no float math) into
pil[i, row, l, tap]; the device exponentiates chunks of slots on the Scalar
engine, pipelined ahead of the DP loop.  Every 8 steps the running sum is
folded out through a fast approximate reciprocal whose exact value is shipped
to the host, so the rescale cancels exactly in the final log-domain combine.
"""

import ml_dtypes
import numpy as np

import concourse.bacc as bacc
import concourse.bass as bass
import concourse.mybir as mybir
import concourse.tile as tile
from concourse.bass_utils import run_bass_kernel_spmd

T = 160
B = 64
C = 6625
S = 25
L = 2 * S + 1  # 51
NCORES = 8
BLOC = B // NCORES  # 8 samples per core
ROWS = 2 * BLOC  # fwd + bwd chains
NITER = 81  # coefficient slots i=0..80; DP loop runs i=1..80
NORM_EVERY = 16
NNORM = 5  # rescales at i = 15, 31, ..., 79
NEG = -1e30
CHUNKS = [4, 8, 12, 17, 20, 20]  # slot-chunk sizes for the DMA/exp pipeline

F32 = mybir.dt.float32
BF16 = mybir.dt.bfloat16
ALU = mybir.AluOpType
ACTF = mybir.ActivationFunctionType
AXIS = mybir.AxisListType


def build_nc() -> bass.Bass:
    nc = bacc.Bacc("TRN2", target_bir_lowering=False)

    pil_d = nc.dram_tensor("pil", [NITER, ROWS, L * 3], F32, kind="ExternalInput")
    oneh_d = nc.dram_tensor("onehotr", [BLOC, L], BF16, kind="ExternalInput")
    out_d = nc.dram_tensor("outv", [ROWS, NNORM + 2], F32, kind="ExternalOutput")
    st_d = nc.dram_tensor("state", [ROWS, L], BF16, kind="ExternalOutput")

    with tile.TileContext(nc) as tc:
        with (
            tc.tile_pool(name="big", bufs=1) as bigp,
            tc.tile_pool(name="small", bufs=1) as smallp,
            tc.tile_pool(name="tmp", bufs=2) as tmpp,
        ):
            pilin = bigp.tile([ROWS, NITER, L * 3], F32, tag="pilin")
            PPQ = bigp.tile([ROWS, NITER, L * 3], BF16, tag="PPQ")

            X = smallp.tile([ROWS, L + 2], BF16, tag="X")
            Y = smallp.tile([ROWS, L + 2], BF16, tag="Y")
            rall = smallp.tile([ROWS, NNORM + 2], F32, tag="rall")
            ssum = smallp.tile([ROWS, NNORM], F32, tag="ssum")
            dummy = smallp.tile([1, 1], F32, tag="dummy")

            # Hoist the Exp act-table load to t~0: a dependency-free dummy
            # activation makes the scheduler place ACT_TABLE_LOAD before any
            # data arrives.
            nc.vector.memset(dummy[:, :], 0.0)
            nc.scalar.activation(dummy[:, :], dummy[:, :], ACTF.Exp)

            nc.vector.memset(X[:, :], 0.0)
            nc.vector.memset(Y[:, :], 0.0)
            nc.vector.memset(rall[:, :], 0.0)

            # bwd init: G_0[sig] = reversed end-state indicator (ACT queue so
            # it overlaps the chunk-0 coefficient DMA on the SP queue).
            nc.scalar.dma_start(out=X[BLOC:ROWS, 2 : L + 2], in_=oneh_d[:, :])

            # Coefficient pipeline: DMA slot-chunk -> exp on ACT.
            s0 = 0
            for ns in CHUNKS:
                in_ap = bass.AP(
                    pil_d,
                    s0 * ROWS * L * 3,
                    [[L * 3, ROWS], [ROWS * L * 3, ns], [1, L * 3]],
                )
                nc.sync.dma_start(out=pilin[:, s0 : s0 + ns, :], in_=in_ap)
                nc.scalar.activation(
                    PPQ[:, s0 : s0 + ns, :], pilin[:, s0 : s0 + ns, :], ACTF.Exp
                )
                s0 += ns

            # fwd init: alpha_0[s=0,1] = p(t=0, l=0,1) = exp'd slot-0 tap 1.
            a0 = PPQ[0:BLOC, 0, :]
            a0v = bass.AP(a0.tensor, a0.offset + 1, [a0.ap[0], [3, 2]])
            nc.vector.tensor_copy(X[0:BLOC, 2:4], a0v)

            cur, nxt = X, Y
            apply_norm = False
            for i in range(1, NITER):
                pp = PPQ[:, i, :]
                ppq_i = bass.AP(pp.tensor, pp.offset, [pp.ap[0], [3, L], [1, 3]])
                xap = cur[:, :]
                xxx = bass.AP(xap.tensor, xap.offset, [xap.ap[0], [1, L], [1, 3]])

                M = tmpp.tile([ROWS, L, 3], BF16, tag="M")
                is_norm = i % NORM_EVERY == NORM_EVERY - 1
                k = i // NORM_EVERY
                if apply_norm or is_norm:
                    nc.vector.scalar_tensor_tensor(
                        out=M[:, :, :],
                        in0=xxx,
                        scalar=rall[:, k - 1 : k] if apply_norm else 1.0,
                        in1=ppq_i,
                        op0=ALU.mult,
                        op1=ALU.mult,
                        accum_out=ssum[:, k : k + 1] if is_norm else None,
                    )
                    apply_norm = False
                else:
                    nc.vector.tensor_tensor(
                        out=M[:, :, :], in0=xxx, in1=ppq_i, op=ALU.mult
                    )
                with nc.allow_low_precision(reason="bf16 DP state; rel tol 2e-2"):
                    nc.vector.tensor_reduce(
                        out=nxt[:, 2 : L + 2], in_=M[:, :, :], axis=AXIS.X, op=ALU.add
                    )
                if is_norm:
                    nc.vector.reciprocal_approx_fast(
                        out=rall[:, k : k + 1], in_=ssum[:, k : k + 1]
                    )
                    apply_norm = True
                cur, nxt = nxt, cur

            # Ship the meeting-point states; the host computes the tiny
            # 51-element dot(alpha_80, reverse(beta_80)) per sample (the
            # pairing crosses partitions, which DVE lanes cannot).
            nc.sync.dma_start(out=st_d[:, :], in_=cur[:, 2 : L + 2])
            nc.scalar.dma_start(out=out_d[:, :], in_=rall[:, :])

    nc.finalize()
    return nc


def host_prep(predictions, targets, target_lengths):
    """Host-side shard + coefficient-layout prep (gather/select/reverse only;
    all float math on the predictions happens on device). Returns per-core
    input maps."""
    predictions = np.asarray(predictions, dtype=np.float32)
    targets = np.asarray(targets)
    target_lengths = np.asarray(target_lengths)

    ext = np.zeros((B, L), dtype=np.int64)
    ext[:, 1::2] = targets
    skip = np.zeros((B, L), dtype=bool)
    skip[:, 3::2] = targets[:, 1:] != targets[:, :-1]

    # sc[t, b, l] = predictions[t, b, ext[b, l]]; q = skip-masked scores
    sc = np.take_along_axis(
        predictions, np.broadcast_to(ext[None], (T, B, L)), axis=2
    )
    q = np.where(skip[None], sc, np.float32(NEG))

    pil = np.full((NITER, 2 * B, L, 3), NEG, dtype=np.float32)
    # fwd rows (slot i = time t=i): taps (q[l], p[l], p[l])
    pil[:, :B, :, 0] = q[:NITER]
    pil[:, :B, :, 1] = sc[:NITER]
    pil[:, :B, :, 2] = sc[:NITER]
    # bwd rows (slot i = time t=160-i, i=1..79), state axis reversed:
    # taps at sigma: (q[52-sig], p[51-sig], p[50-sig]); l out of range -> NEG
    sig = np.arange(L)
    for i in range(1, NITER - 1):
        t = T - i
        l0, l1, l2 = 52 - sig, 51 - sig, 50 - sig
        v0 = np.where(l0[None] < L, q[t][:, np.minimum(l0, L - 1)], np.float32(NEG))
        v1 = np.where(l1[None] < L, sc[t][:, np.minimum(l1, L - 1)], np.float32(NEG))
        pil[i, B:, :, 0] = v0
        pil[i, B:, :, 1] = v1
        pil[i, B:, :, 2] = sc[t][:, l2]
    # bwd slot 80: identity step (taps exp -> (0, 0, 1))
    pil[NITER - 1, B:, :, 2] = 0.0

    idx = (2 * target_lengths).astype(np.int64)
    oneh_rev = np.zeros((B, L), dtype=np.float32)
    oneh_rev[np.arange(B), (L - 1) - idx] = 1.0
    oneh_rev[np.arange(B), (L - 1) - (idx - 1)] = 1.0

    in_maps = []
    for c in range(NCORES):
        bsl = slice(c * BLOC, (c + 1) * BLOC)
        bsl2 = slice(B + c * BLOC, B + (c + 1) * BLOC)
        pshard = np.concatenate([pil[:, bsl], pil[:, bsl2]], axis=1)  # [81,16,51,3]
        in_maps.append(
            {
                "pil": np.ascontiguousarray(pshard).reshape(NITER, ROWS, L * 3),
                "onehotr": oneh_rev[bsl].astype(ml_dtypes.bfloat16),
            }
        )
    return in_maps


_NC_CACHE = {}


def kernel(predictions, targets, target_lengths):
    if "nc" not in _NC_CACHE:
        _NC_CACHE["nc"] = build_nc()
    nc = _NC_CACHE["nc"]

    in_maps = host_prep(predictions, targets, target_lengths)
    res = run_bass_kernel_spmd(nc, in_maps, core_ids=list(range(NCORES)))
    return finish(res.results, target_lengths)


def finish(results, target_lengths):
    nlls = []
    with np.errstate(divide="ignore"):
        for r in results:
            outv = r["outv"].reshape(ROWS, NNORM + 2)
            st = r["state"].reshape(ROWS, L).astype(np.float32)
            slog = -np.log(outv[:, :NNORM]).sum(axis=1)  # -sum log rcol per row
            dot = (st[:BLOC] * st[BLOC:, ::-1]).sum(axis=1)
            nlls.append(-(np.log(dot) + slog[:BLOC] + slog[BLOC:]))
    nll = np.concatenate(nlls).astype(np.float32)
    lengths = np.asarray(target_lengths).astype(np.float32)
    per = np.where(nll >= 1e29, np.float32(0.0), nll / lengths)
    return np.array(per.mean(), dtype=np.float32)
